# revision 1
# baseline (speedup 1.0000x reference)
"""Distributed multi-head attention kernel for 8 TRN2 NeuronCores.

Problem: B=2, N=2048, C=1024, H=16 heads, D=64.
  out = softmax((q@Wq)(k@Wk)^T / sqrt(D)) @ (v@Wv) @ Wo   (per head, biases are zero)

Sharding: sequence-parallel within batch (2 batch groups x 4 cores), with NO
collectives: measured AllGather on this fleet has a ~65us fixed cost plus slow
streaming (~110-145us total), while recomputing the full-batch K/V projections
locally costs ~109us of perfectly parallel PE time and keeps every core
independent.  Core c owns batch b=c//4, query rows R=[512r, 512r+512), r=c%4.

Per-core dataflow (all PE inputs bf16, PSUM/softmax f32):
  1. Q^T = Wq^T @ xq^T  (own rows; channels on partitions; stays in SBUF)
  2. K^T = Wk^T @ xk^T  for the FULL batch -> resident SBUF (128, 8*2048)
     S^T(pair i) = K @ Q^T interleaved right after K^T; exp on ScalarE
     (scale=1/8 folded in; no max-subtraction needed for ~N(0,1) scores)
  3. V' = [xv @ Wv | ones] full batch, per-head 65-col groups -> resident SBUF
  4. O'^T = V'^T @ P^T accumulated over key chunks; row 64 = softmax denom.
     Normalize with reciprocal broadcast; accumulate A^T in SBUF.
     PV(i) is software-pipelined against S^T(i+1) so the PE never waits on exp.
  5. out^T = Wo^T @ A^T -> DRAM (f32). Host transposes + concatenates.
"""

import sys

sys.path.insert(0, "/opt/trn_rl_repo")

from contextlib import ExitStack

import numpy as np
import ml_dtypes

import concourse.bass as bass
import concourse.bacc as bacc
import concourse.mybir as mybir
import concourse.tile as tile
from concourse.bass_utils import run_bass_kernel_spmd

BF16 = mybir.dt.bfloat16
F32 = mybir.dt.float32
Exp = mybir.ActivationFunctionType.Exp

B, N, C = 2, 2048, 1024
H, D = 16, 64
DV = D + 1          # V columns per head incl. ones column
NQ = N // 4         # queries per core = 512
NCHUNK = N // 128   # 16 key chunks
SCALE = 1.0 / np.sqrt(D)
PRELUDE = 1         # S^T pairs emitted before the V' projection phase

_CACHE = {}


def build_nc():
    nc = bacc.Bacc("TRN2", target_bir_lowering=False, debug=False, num_devices=8)

    xqT = nc.declare_dram_parameter("xqT", [C, NQ], BF16, isOutput=False)
    xkT = nc.declare_dram_parameter("xkT", [C, N], BF16, isOutput=False)
    xvT = nc.declare_dram_parameter("xvT", [C, N], BF16, isOutput=False)
    wq = nc.declare_dram_parameter("wq", [C, C], BF16, isOutput=False)
    wk = nc.declare_dram_parameter("wk", [C, C], BF16, isOutput=False)
    wv = nc.declare_dram_parameter("wv", [C, C], BF16, isOutput=False)
    wo = nc.declare_dram_parameter("wo", [C, C], BF16, isOutput=False)
    outT = nc.declare_dram_parameter("outT", [C, NQ], F32, isOutput=True)

    with tile.TileContext(nc) as tc, ExitStack() as top:
        # ---------------- resident SBUF (~83 KB/partition) ----------------
        res = top.enter_context(tc.tile_pool(name="res", bufs=1))
        qT_sb = res.tile([128, 8 * NQ], BF16, tag="qT")     # Q^T: pair i at cols 512i
        kT_sb = res.tile([128, 8 * N], BF16, tag="kT")      # K^T: pair i at cols 2048i
        v1_sb = res.tile([128, NCHUNK * H * DV], BF16, tag="v1")  # V' chunk kc at 1040*kc
        aT_sb = res.tile([128, 8 * NQ], BF16, tag="aT")     # A^T accum
        dinv_sb = res.tile([64, NQ], F32, tag="dinv")
        drow_sb = res.tile([1, NQ], F32, tag="drow")
        draw_sb = res.tile([1, NQ], F32, tag="draw")

        attn_stack = ExitStack()

        def st_pair(i):
            """Emit S^T + exp for head pair i; returns the P tile."""
            Pp = P_pool.tile([128, NCHUNK * 1024], BF16, tag="P", name=f"P_{i}")
            for kc in range(NCHUNK):
                st = spool.tile([128, 1024], F32, tag="st", name=f"st_{i}_{kc}")
                key_sl = kT_sb[:, N * i + 128 * kc:N * i + 128 * (kc + 1)]
                nc.tensor.matmul(st[:, 0:512],
                                 key_sl[0:64, :],
                                 qT_sb[0:64, NQ * i:NQ * (i + 1)],
                                 start=True, stop=True)
                nc.tensor.matmul(st[:, 512:1024],
                                 key_sl[64:128, :],
                                 qT_sb[64:128, NQ * i:NQ * (i + 1)],
                                 start=True, stop=True)
                nc.scalar.activation(Pp[:, 1024 * kc:1024 * (kc + 1)], st[:],
                                     Exp, scale=float(SCALE))
            return Pp

        with ExitStack() as ph:
            wpool = ph.enter_context(tc.tile_pool(name="wpool", bufs=9))
            xqpool = ph.enter_context(tc.tile_pool(name="xqpool", bufs=8))
            xkpool = ph.enter_context(tc.tile_pool(name="xkpool", bufs=8))
            ppool = ph.enter_context(tc.tile_pool(name="ppool", bufs=8, space="PSUM"))

            # ---------------- Q^T projection ----------------
            # cc-streamed: psum[m 0-3] and [4-7] accumulate as (wq[cc], xq[cc])
            # arrive, so the first matmul only waits on the cc=0 DMAs.
            xq_t, wq_t = [], []
            for cc in range(8):
                xq_t.append(xqpool.tile([128, NQ], BF16, tag="xq", name=f"xq_t{cc}"))
                nc.sync.dma_start(out=xq_t[cc][:], in_=xqT[128 * cc:128 * (cc + 1), :])
                wq_t.append(wpool.tile([128, C], BF16, tag="w", name=f"wq_t{cc}"))
                nc.sync.dma_start(out=wq_t[cc][:], in_=wq[128 * cc:128 * (cc + 1), :])
            for mg in range(2):
                ps = [ppool.tile([128, NQ], F32, tag="ps", name=f"qps{mg}_{m}")
                      for m in range(4)]
                for cc in range(8):
                    for m in range(4):
                        nc.tensor.matmul(ps[m][:],
                                         wq_t[cc][:, 128 * (4 * mg + m):
                                                   128 * (4 * mg + m + 1)],
                                         xq_t[cc][:],
                                         start=(cc == 0), stop=(cc == 7))
                for m in range(4):
                    nc.vector.tensor_copy(
                        qT_sb[:, NQ * (4 * mg + m):NQ * (4 * mg + m + 1)], ps[m][:])

            # ------------- K^T projection (full batch) -------------
            # lhsT (wk slice) reused across the 4 query blocks per ldweights.
            wk_t, xk_t = [], []
            for cc in range(8):
                wk_t.append(wpool.tile([128, C], BF16, tag="w", name=f"wk_t{cc}"))
                nc.sync.dma_start(out=wk_t[cc][:], in_=wk[128 * cc:128 * (cc + 1), :])
                xk_t.append(xkpool.tile([128, N], BF16, tag="xk", name=f"xk_t{cc}"))
                nc.sync.dma_start(out=xk_t[cc][:], in_=xkT[128 * cc:128 * (cc + 1), :])
            for m in range(8):
                ps = [ppool.tile([128, 512], F32, tag="ps", name=f"kps{m}_{qb}")
                      for qb in range(4)]
                for cc in range(8):
                    for qb in range(4):
                        nc.tensor.matmul(ps[qb][:],
                                         wk_t[cc][:, 128 * m:128 * (m + 1)],
                                         xk_t[cc][:, 512 * qb:512 * (qb + 1)],
                                         start=(cc == 0), stop=(cc == 7))
                for qb in range(4):
                    nc.vector.tensor_copy(
                        kT_sb[:, N * m + 512 * qb:N * m + 512 * (qb + 1)], ps[qb][:])

        spool = attn_stack.enter_context(
            tc.tile_pool(name="spool", bufs=3, space="PSUM"))       # 6 banks
        P_pool = attn_stack.enter_context(
            tc.tile_pool(name="P_pool", bufs=PRELUDE + 1))          # 32 KB each

        P_tiles = {}

        # ---------------- V' projection (full batch) ----------------
        # psum comes from spool (one (128,1024) tile per key chunk, both
        # halves); lhsT (xv chunk) reused across both halves per ldweights.
        with ExitStack() as ph:
            wpool = ph.enter_context(tc.tile_pool(name="wvpool", bufs=9))
            xvpool = ph.enter_context(tc.tile_pool(name="xvpool", bufs=8))
            wv_t, xv_t = [], []
            for cc in range(8):
                wv_t.append(wpool.tile([128, C], BF16, tag="w", name=f"wv_t{cc}"))
                nc.sync.dma_start(out=wv_t[cc][:], in_=wv[128 * cc:128 * (cc + 1), :])
                xv_t.append(xvpool.tile([128, N], BF16, tag="xv", name=f"xv_t{cc}"))
                nc.sync.dma_start(out=xv_t[cc][:], in_=xvT[128 * cc:128 * (cc + 1), :])
            v3 = v1_sb[:].rearrange("p (kc h x) -> p kc h x", kc=NCHUNK, x=DV)
            nc.vector.memset(v3[:, :, :, D:DV], 1.0)
            Pp0 = P_pool.tile([128, NCHUNK * 1024], BF16, tag="P", name="P_0")
            P_tiles[0] = Pp0
            for kc in range(NCHUNK):
                ps = spool.tile([128, 1024], F32, tag="st", name=f"vps{kc}")
                for cc in range(8):
                    for half in range(2):
                        nc.tensor.matmul(
                            ps[:, 512 * half:512 * (half + 1)],
                            xv_t[cc][:, 128 * kc:128 * (kc + 1)],
                            wv_t[cc][:, 512 * half:512 * (half + 1)],
                            start=(cc == 0), stop=(cc == 7))
                for half in range(2):
                    nc.vector.tensor_copy(
                        v3[:, kc, 8 * half:8 * (half + 1), 0:D],
                        ps[:, 512 * half:512 * (half + 1)]
                        .rearrange("p (h d) -> p h d", d=D))
                st = spool.tile([128, 1024], F32, tag="st", name=f"st0_{kc}")
                key_sl = kT_sb[:, 128 * kc:128 * (kc + 1)]
                nc.tensor.matmul(st[:, 0:512], key_sl[0:64, :],
                                 qT_sb[0:64, 0:NQ], start=True, stop=True)
                nc.tensor.matmul(st[:, 512:1024], key_sl[64:128, :],
                                 qT_sb[64:128, 0:NQ], start=True, stop=True)
                nc.scalar.activation(Pp0[:, 1024 * kc:1024 * (kc + 1)], st[:],
                                     Exp, scale=float(SCALE))

        # ---------- attention: PV(i) pipelined against S^T(i+PRELUDE) ----------
        with ExitStack() as ph:
            opool = ph.enter_context(tc.tile_pool(name="opool", bufs=2, space="PSUM"))
            wopool = ph.enter_context(tc.tile_pool(name="wopool", bufs=8))
            wo_t = [wopool.tile([128, C], BF16, tag="wo", name=f"wo_t{j}")
                    for j in range(8)]
            for cc in range(8):
                nc.sync.dma_start(out=wo_t[cc][:], in_=wo[128 * cc:128 * (cc + 1), :])
            for i in range(8):
                Pp = P_tiles.pop(i)
                if i + PRELUDE < 8:
                    P_tiles[i + PRELUDE] = st_pair(i + PRELUDE)
                for h in range(2):
                    po = opool.tile([128, NQ], F32, tag="po", name=f"po{i}_{h}")
                    for kc in range(NCHUNK):
                        nc.tensor.matmul(
                            po[0:DV, :],
                            v1_sb[:].rearrange("p (kc v) -> p kc v", v=H * DV)
                                 [:, kc, 2 * DV * i + DV * h:2 * DV * i + DV * (h + 1)],
                            Pp[:, 1024 * kc + 512 * h:1024 * kc + 512 * h + 512],
                            start=(kc == 0), stop=(kc == NCHUNK - 1))
                    nc.vector.tensor_copy(draw_sb[:], po[D:DV, :])
                    nc.vector.reciprocal_approx_fast(drow_sb[:], draw_sb[:])
                    nc.gpsimd.partition_broadcast(dinv_sb[:], drow_sb[:])
                    nc.vector.tensor_mul(
                        aT_sb[64 * h:64 * (h + 1), NQ * i:NQ * (i + 1)],
                        po[0:D, :], dinv_sb[:])

            # ---------------- output projection ----------------
            epool = ph.enter_context(tc.tile_pool(name="eopool", bufs=3))
            for m in range(8):
                ps = opool.tile([128, NQ], F32, tag="po", name=f"ops{m}")
                for cc in range(8):
                    nc.tensor.matmul(ps[:], wo_t[cc][:, 128 * m:128 * (m + 1)],
                                     aT_sb[:, NQ * cc:NQ * (cc + 1)],
                                     start=(cc == 0), stop=(cc == 7))
                ev = epool.tile([128, NQ], F32, tag="ev", name=f"oev{m}")
                nc.vector.tensor_copy(ev[:], ps[:])
                nc.sync.dma_start(out=outT[128 * m:128 * (m + 1), :], in_=ev[:])
        attn_stack.close()

    nc.compile()
    return nc


def _get_nc():
    if "nc" not in _CACHE:
        _CACHE["nc"] = build_nc()
    return _CACHE["nc"]


def _make_in_maps(q, k, v, Wq, Wk, Wv, Wo):
    bf = ml_dtypes.bfloat16
    wq_b = np.ascontiguousarray(Wq).astype(bf)
    wk_b = np.ascontiguousarray(Wk).astype(bf)
    wv_b = np.ascontiguousarray(Wv).astype(bf)
    wo_b = np.ascontiguousarray(Wo).astype(bf)
    q = np.asarray(q)
    kT = [np.ascontiguousarray(np.asarray(k)[b].T).astype(bf) for b in range(B)]
    vT = [np.ascontiguousarray(np.asarray(v)[b].T).astype(bf) for b in range(B)]
    in_maps = []
    for c in range(8):
        b, r = c // 4, c % 4
        sl = slice(NQ * r, NQ * (r + 1))
        in_maps.append({
            "xqT": np.ascontiguousarray(q[b, sl, :].T).astype(bf),
            "xkT": kT[b], "xvT": vT[b],
            "wq": wq_b, "wk": wk_b, "wv": wv_b, "wo": wo_b,
        })
    return in_maps


def _run(inputs, trace=False, **kw):
    nc = _get_nc()
    in_maps = _make_in_maps(inputs["q"], inputs["k"], inputs["v"],
                            inputs["Wq"], inputs["Wk"], inputs["Wv"], inputs["Wo"])
    res = None
    for attempt in range(3):
        try:
            res = run_bass_kernel_spmd(nc, in_maps, core_ids=list(range(8)),
                                       trace=trace, **kw)
            break
        except Exception:
            if attempt == 2:
                raise
            import time
            time.sleep(2.0)
    out = np.empty((B, N, C), np.float32)
    for c in range(8):
        b, r = c // 4, c % 4
        out[b, NQ * r:NQ * (r + 1), :] = res.results[c]["outT"].T
    return out, res


def kernel(**inputs) -> np.ndarray:
    out, _ = _run(inputs, trace=False)
    return out



# revision 2
# speedup vs baseline: 1.1931x; 1.1931x over previous
"""Distributed multi-head attention kernel for 8 TRN2 NeuronCores.

Problem: B=2, N=2048, C=1024, H=16 heads, D=64.
  out = softmax((q@Wq)(k@Wk)^T / sqrt(D)) @ (v@Wv) @ Wo   (biases are zero)

Sharding (hybrid, per the hint): data-parallel over batch x tensor-parallel
over heads.  Core c owns batch b=c//4 and head group g=c%4 (4 heads = 256
channels).  Each core projects Q/K/V only for its own 256 columns over the
full 2048-token sequence, runs attention for its 4 heads, then multiplies by
its 256 rows of Wo producing a PARTIAL (1024, 2048) output.  The host sums
the 4 partials per batch (the out_proj all-reduce) and transposes.  Zero
redundant FLOPs: 2.15 GMAC/core vs 5.9 GMAC/core for the old seq-parallel
scheme that recomputed full K/V projections per core.

Per-core dataflow (PE inputs bf16, PSUM/softmax f32):
  1. K^T = Wk_g^T x_k^T, Q^T = Wq_g^T x_q^T  (channels on partitions, SBUF)
  2. V' = [x_v @ Wv_g | ones] per-head 65-col groups, resident SBUF
  3. per unit u=(qb,hp) (4 query blocks x 2 head pairs): S^T = K Q^T via
     row-tiled concurrent matmul pairs (head even on partitions 0:64, head
     odd on 64:128); exp on ScalarE (scale=1/8 folded; no max-subtraction
     needed for ~N(0,1) scores).  ScalarE is the critical path (~147us of
     ACTIVATE) so everything else is interleaved into its shadow:
     V' proj inside scores(u0), PV(u) inside scores(u+1), out_proj(qb)
     right after each odd unit.
  4. PV: O'^T = V'^T P^T accumulated over 16 key chunks; row 64 = softmax
     denominator; normalize via reciprocal+broadcast; A^T accum in SBUF.
  5. out^T_partial = Wo_g^T A^T -> DRAM f32 per query block.
"""

import sys

sys.path.insert(0, "/opt/trn_rl_repo")

from contextlib import ExitStack

import numpy as np
import ml_dtypes

import concourse.bass as bass
import concourse.bacc as bacc
import concourse.mybir as mybir
import concourse.tile as tile
from concourse.bass_utils import run_bass_kernel_spmd

BF16 = mybir.dt.bfloat16
F32 = mybir.dt.float32
Exp = mybir.ActivationFunctionType.Exp

B, N, C = 2, 2048, 1024
H, D = 16, 64
HL, CL = 4, 256     # heads / channels owned per core
DV = D + 1          # V' columns per head incl. ones column
NCH = N // 128      # 16 key chunks
NQB = 4             # query blocks of 512
SCALE = 1.0 / np.sqrt(D)

_CACHE = {}


def build_nc():
    nc = bacc.Bacc("TRN2", target_bir_lowering=False, debug=False, num_devices=8)

    xqT = nc.declare_dram_parameter("xqT", [C, N], BF16, isOutput=False)
    xkT = nc.declare_dram_parameter("xkT", [C, N], BF16, isOutput=False)
    xvT = nc.declare_dram_parameter("xvT", [C, N], BF16, isOutput=False)
    wq = nc.declare_dram_parameter("wq", [C, CL], BF16, isOutput=False)
    wk = nc.declare_dram_parameter("wk", [C, CL], BF16, isOutput=False)
    wv = nc.declare_dram_parameter("wv", [C, CL], BF16, isOutput=False)
    wo = nc.declare_dram_parameter("wo", [CL, C], BF16, isOutput=False)
    outT = nc.declare_dram_parameter("outT", [C, N], F32, isOutput=True)

    with tile.TileContext(nc) as tc, ExitStack() as top:
        res = top.enter_context(tc.tile_pool(name="res", bufs=1))
        qT_sb = res.tile([128, 2 * N], BF16, tag="qT")   # chunk hp at cols N*hp
        kT_sb = res.tile([128, 2 * N], BF16, tag="kT")
        v1_sb = res.tile([128, NCH * HL * DV], BF16, tag="v1")
        aT_sb = res.tile([128, 2 * N], BF16, tag="aT")

        # ---------------- phase 1: K then Q projections ----------------
        with ExitStack() as ph:
            wqk = ph.enter_context(tc.tile_pool(name="wqk", bufs=16))
            xqk = ph.enter_context(tc.tile_pool(name="xqk", bufs=16))
            qkps = ph.enter_context(tc.tile_pool(name="qkps", bufs=4, space="PSUM"))

            wk_t, xk_t, wq_t, xq_t = [], [], [], []
            for cc in range(8):
                wk_t.append(wqk.tile([128, CL], BF16, tag="w", name=f"wk{cc}"))
                nc.sync.dma_start(out=wk_t[cc][:], in_=wk[128 * cc:128 * (cc + 1), :])
                xk_t.append(xqk.tile([128, N], BF16, tag="x", name=f"xk{cc}"))
                nc.sync.dma_start(out=xk_t[cc][:], in_=xkT[128 * cc:128 * (cc + 1), :])
            for cc in range(8):
                wq_t.append(wqk.tile([128, CL], BF16, tag="w", name=f"wq{cc}"))
                nc.sync.dma_start(out=wq_t[cc][:], in_=wq[128 * cc:128 * (cc + 1), :])
                xq_t.append(xqk.tile([128, N], BF16, tag="x", name=f"xq{cc}"))
                nc.sync.dma_start(out=xq_t[cc][:], in_=xqT[128 * cc:128 * (cc + 1), :])

            def proj_qk(dst_sb, w_t, x_t, nm):
                for hp in range(2):
                    ps = [qkps.tile([128, 512], F32, tag="ps", name=f"{nm}ps{hp}_{qb}")
                          for qb in range(NQB)]
                    for cc in range(8):
                        for qb in range(NQB):
                            nc.tensor.matmul(ps[qb][:],
                                             w_t[cc][:, 128 * hp:128 * (hp + 1)],
                                             x_t[cc][:, 512 * qb:512 * (qb + 1)],
                                             start=(cc == 0), stop=(cc == 7))
                    for qb in range(NQB):
                        nc.vector.tensor_copy(
                            dst_sb[:, N * hp + 512 * qb:N * hp + 512 * (qb + 1)],
                            ps[qb][:])

            proj_qk(kT_sb, wk_t, xk_t, "k")
            proj_qk(qT_sb, wq_t, xq_t, "q")

        # ---------------- attention-phase pools ----------------
        P_pool = top.enter_context(tc.tile_pool(name="Ppool", bufs=2))
        spool = top.enter_context(tc.tile_pool(name="spool", bufs=3, space="PSUM"))
        popool = top.enter_context(tc.tile_pool(name="popool", bufs=2, space="PSUM"))
        wopool = top.enter_context(tc.tile_pool(name="wopool", bufs=2))
        evpool = top.enter_context(tc.tile_pool(name="evpool", bufs=3))
        dpool = top.enter_context(tc.tile_pool(name="dpool", bufs=2))

        wo_t = []
        for lc in range(2):
            wo_t.append(wopool.tile([128, C], BF16, tag="wo", name=f"wo{lc}"))
            nc.sync.dma_start(out=wo_t[lc][:], in_=wo[128 * lc:128 * (lc + 1), :])

        v4 = v1_sb[:].rearrange("p (kc h x) -> p kc h x", kc=NCH, x=DV)

        # units: u = 2*qb + hp; scores(u) emits S^T+exp for head pair hp,
        # query block qb.  fill_cb(kc) interleaves PE work into the stream.
        def emit_scores_unit(u, fill_cb=None):
            qb, hp = u // 2, u % 2
            Pp = P_pool.tile([128, NCH * 1024], BF16, tag="P", name=f"P{u}")
            for kc in range(NCH):
                st = spool.tile([128, 1024], F32, tag="st", name=f"st{u}_{kc}")
                ksl = kT_sb[:, N * hp + 128 * kc:N * hp + 128 * (kc + 1)]
                q0 = qT_sb[0:64, N * hp + 512 * qb:N * hp + 512 * (qb + 1)]
                q1 = qT_sb[64:128, N * hp + 512 * qb:N * hp + 512 * (qb + 1)]
                nc.tensor.matmul(st[:, 0:512], ksl[0:64, :], q0,
                                 start=True, stop=True)
                nc.tensor.matmul(st[:, 512:1024], ksl[64:128, :], q1,
                                 start=True, stop=True)
                nc.scalar.activation(Pp[:, 1024 * kc:1024 * (kc + 1)], st[:],
                                     Exp, scale=float(SCALE))
                if fill_cb is not None:
                    fill_cb(kc)
            return Pp

        # ---- V' projection: interleaved into scores(u0) ----
        ph2 = ExitStack()
        wvp = ph2.enter_context(tc.tile_pool(name="wvp", bufs=8))
        xvp = ph2.enter_context(tc.tile_pool(name="xvp", bufs=8))
        wv_t, xv_t = [], []
        for cc in range(8):
            wv_t.append(wvp.tile([128, CL], BF16, tag="wv", name=f"wv{cc}"))
            nc.sync.dma_start(out=wv_t[cc][:], in_=wv[128 * cc:128 * (cc + 1), :])
            xv_t.append(xvp.tile([128, N], BF16, tag="xv", name=f"xv{cc}"))
            nc.sync.dma_start(out=xv_t[cc][:], in_=xvT[128 * cc:128 * (cc + 1), :])
        nc.vector.memset(v4[:, :, :, D:DV], 1.0)

        def vproj_cb(kc):
            ps = popool.tile([128, 512], F32, tag="po", name=f"vps{kc}")
            for cc in range(8):
                nc.tensor.matmul(ps[:, 0:CL],
                                 xv_t[cc][:, 128 * kc:128 * (kc + 1)],
                                 wv_t[cc][:],
                                 start=(cc == 0), stop=(cc == 7))
            nc.vector.tensor_copy(
                v4[:, kc, :, 0:D],
                ps[:, 0:CL].rearrange("p (h d) -> p h d", d=D))

        P_cur = emit_scores_unit(0, vproj_cb)
        ph2.close()

        # ---- PV(u) + normalize; interleaved into scores(u+1) ----
        def make_pv_cb(u, Pp):
            qb, hp = u // 2, u % 2
            po_e = popool.tile([128, 512], F32, tag="po", name=f"poe{u}")
            po_o = popool.tile([128, 512], F32, tag="po", name=f"poo{u}")

            def cb(kc):
                nc.tensor.matmul(po_e[0:DV, :], v4[:, kc, 2 * hp, :],
                                 Pp[:, 1024 * kc:1024 * kc + 512],
                                 start=(kc == 0), stop=(kc == NCH - 1))
                nc.tensor.matmul(po_o[0:DV, :], v4[:, kc, 2 * hp + 1, :],
                                 Pp[:, 1024 * kc + 512:1024 * (kc + 1)],
                                 start=(kc == 0), stop=(kc == NCH - 1))

            def fin():
                for par, po in ((0, po_e), (1, po_o)):
                    draw = dpool.tile([1, 512], F32, tag="draw", name=f"dr{u}_{par}")
                    drow = dpool.tile([1, 512], F32, tag="drow", name=f"dw{u}_{par}")
                    dinv = dpool.tile([64, 512], F32, tag="dinv", name=f"di{u}_{par}")
                    nc.vector.tensor_copy(draw[:], po[D:DV, :])
                    nc.vector.reciprocal_approx_fast(drow[:], draw[:])
                    nc.gpsimd.partition_broadcast(dinv[:], drow[:])
                    nc.vector.tensor_mul(
                        aT_sb[64 * par:64 * (par + 1),
                              N * hp + 512 * qb:N * hp + 512 * (qb + 1)],
                        po[0:D, :], dinv[:])

            return cb, fin

        def emit_outproj(qb):
            for m in range(8):
                ps = popool.tile([128, 512], F32, tag="po", name=f"ops{qb}_{m}")
                for lc in range(2):
                    nc.tensor.matmul(ps[:], wo_t[lc][:, 128 * m:128 * (m + 1)],
                                     aT_sb[:, N * lc + 512 * qb:N * lc + 512 * (qb + 1)],
                                     start=(lc == 0), stop=(lc == 1))
                ev = evpool.tile([128, 512], F32, tag="ev", name=f"oev{qb}_{m}")
                nc.vector.tensor_copy(ev[:], ps[:])
                nc.sync.dma_start(out=outT[128 * m:128 * (m + 1),
                                           512 * qb:512 * (qb + 1)], in_=ev[:])

        for u in range(8):
            pv_cb, pv_fin = make_pv_cb(u, P_cur)
            if u + 1 < 8:
                P_cur = emit_scores_unit(u + 1, pv_cb)
            else:
                for kc in range(NCH):
                    pv_cb(kc)
            pv_fin()
            if u % 2 == 1:
                emit_outproj(u // 2)

    nc.compile()
    return nc


def _get_nc():
    if "nc" not in _CACHE:
        _CACHE["nc"] = build_nc()
    return _CACHE["nc"]


def _make_in_maps(q, k, v, Wq, Wk, Wv, Wo):
    bf = ml_dtypes.bfloat16
    q, k, v = np.asarray(q), np.asarray(k), np.asarray(v)
    Wq, Wk, Wv, Wo = (np.asarray(a) for a in (Wq, Wk, Wv, Wo))
    xqT = [np.ascontiguousarray(q[b].T).astype(bf) for b in range(B)]
    xkT = [np.ascontiguousarray(k[b].T).astype(bf) for b in range(B)]
    xvT = [np.ascontiguousarray(v[b].T).astype(bf) for b in range(B)]
    in_maps = []
    for c in range(8):
        b, g = c // 4, c % 4
        cs = slice(CL * g, CL * (g + 1))
        in_maps.append({
            "xqT": xqT[b], "xkT": xkT[b], "xvT": xvT[b],
            "wq": np.ascontiguousarray(Wq[:, cs]).astype(bf),
            "wk": np.ascontiguousarray(Wk[:, cs]).astype(bf),
            "wv": np.ascontiguousarray(Wv[:, cs]).astype(bf),
            "wo": np.ascontiguousarray(Wo[cs, :]).astype(bf),
        })
    return in_maps


def _run(inputs, trace=False, **kw):
    nc = _get_nc()
    in_maps = _make_in_maps(inputs["q"], inputs["k"], inputs["v"],
                            inputs["Wq"], inputs["Wk"], inputs["Wv"], inputs["Wo"])
    res = None
    for attempt in range(3):
        try:
            res = run_bass_kernel_spmd(nc, in_maps, core_ids=list(range(8)),
                                       trace=trace, **kw)
            break
        except Exception:
            if attempt == 2:
                raise
            import time
            time.sleep(2.0)
    out = np.empty((B, N, C), np.float32)
    for b in range(B):
        acc = res.results[4 * b]["outT"].copy()
        for g in range(1, 4):
            acc += res.results[4 * b + g]["outT"]
        out[b] = acc.T
    return out, res


def kernel(**inputs) -> np.ndarray:
    out, _ = _run(inputs, trace=False)
    return out


# revision 4
# speedup vs baseline: 1.2897x; 1.0810x over previous
"""Distributed multi-head attention kernel for 8 TRN2 NeuronCores.

Problem: B=2, N=2048, C=1024, H=16 heads, D=64.
  out = softmax((q@Wq)(k@Wk)^T / sqrt(D)) @ (v@Wv) @ Wo   (biases are zero)

Sharding (hybrid): data-parallel over batch x tensor-parallel over heads.
Core c owns batch b=c//4 and head group g=c%4 (4 heads = 256 channels); it
projects Q/K/V for its own 256 columns over the full sequence, runs
attention for its 4 heads, applies its 256 rows of Wo and writes a PARTIAL
(1024, 2048) bf16 output.  The host sums the 4 partials per batch (the
out_proj all-reduce) and transposes.  Zero redundant FLOPs.

ScalarE is the critical path (~145us of exp ACTIVATE at (1024+352)/1.2 ns
per [128,1024] tile).  The emission strategy keeps it saturated:
  - batched DMAs (one descriptor per tensor half) so the head is
    transfer-bound, not dispatch-bound;
  - every non-score PE task (V' projection, PV, out_proj) is queued as
    small filler items interleaved into the score/exp slot stream, with
    slot gates to avoid in-order PE stalls on not-yet-ready deps;
  - out_proj of query block qb is spread into the unit after fin((qb,hp1));
  - the last unit's PV runs head-even first so its normalize overlaps the
    head-odd chain.
"""

import sys

sys.path.insert(0, "/opt/trn_rl_repo")

from contextlib import ExitStack

import numpy as np
import ml_dtypes

import concourse.bass as bass
import concourse.bacc as bacc
import concourse.mybir as mybir
import concourse.tile as tile
from concourse.bass_utils import run_bass_kernel_spmd

BF16 = mybir.dt.bfloat16
F32 = mybir.dt.float32
Exp = mybir.ActivationFunctionType.Exp

B, N, C = 2, 2048, 1024
H, D = 16, 64
HL, CL = 4, 256     # heads / channels owned per core
DV = D + 1          # V' columns per head incl. ones column
NCH = N // 128      # 16 key chunks
SCALE = 1.0 / np.sqrt(D)

_CACHE = {}


def build_nc():
    nc = bacc.Bacc("TRN2", target_bir_lowering=False, debug=False, num_devices=8)

    xqT = nc.declare_dram_parameter("xqT", [C, N], BF16, isOutput=False)
    xkT = nc.declare_dram_parameter("xkT", [C, N], BF16, isOutput=False)
    xvT = nc.declare_dram_parameter("xvT", [C, N], BF16, isOutput=False)
    wq = nc.declare_dram_parameter("wq", [C, CL], BF16, isOutput=False)
    wk = nc.declare_dram_parameter("wk", [C, CL], BF16, isOutput=False)
    wv = nc.declare_dram_parameter("wv", [C, CL], BF16, isOutput=False)
    wo = nc.declare_dram_parameter("wo", [CL, C], BF16, isOutput=False)
    outT = nc.declare_dram_parameter("outT", [C, N], BF16, isOutput=True)

    with tile.TileContext(nc) as tc, ExitStack() as top:
        res = top.enter_context(tc.tile_pool(name="res", bufs=1))
        qT_sb = res.tile([128, 2 * N], BF16, tag="qT")   # chunk hp at cols N*hp
        kT_sb = res.tile([128, 2 * N], BF16, tag="kT")
        v1_sb = res.tile([128, NCH * HL * DV], BF16, tag="v1")
        aT_sb = res.tile([128, 2 * N], BF16, tag="aT")
        v4 = v1_sb[:].rearrange("p (kc h x) -> p kc h x", kc=NCH, x=DV)

        def dma_in_grouped(dst_tile, src, blocks, tag):
            """One DMA per row-block: src rows [128*b0, 128*b1) -> dst col
            groups, dst laid out [128, nblk*width]."""
            width = src.shape[1]
            dst3 = dst_tile[:].rearrange("p (c m) -> p c m", m=width)
            for b0, b1 in blocks:
                nc.sync.dma_start(
                    out=dst3[:, b0:b1, :],
                    in_=src[128 * b0:128 * b1, :]
                    .rearrange("(c p) m -> p c m", p=128))

        # ---------------- phase A: K and Q projections ----------------
        with ExitStack() as ph:
            wpool = ph.enter_context(tc.tile_pool(name="wpool", bufs=1))
            xpool = ph.enter_context(tc.tile_pool(name="xpool", bufs=1))
            qkps = ph.enter_context(tc.tile_pool(name="qkps", bufs=4, space="PSUM"))

            wk_t = wpool.tile([128, 8 * CL], BF16, tag="wk")
            wq_t = wpool.tile([128, 8 * CL], BF16, tag="wq")
            xk_t = xpool.tile([128, 8 * N], BF16, tag="xk")
            xq_t = xpool.tile([128, 8 * N], BF16, tag="xq")
            dma_in_grouped(wk_t, wk, [(0, 8)], "wk")
            dma_in_grouped(wq_t, wq, [(0, 8)], "wq")
            dma_in_grouped(xk_t, xkT, [(0, 4), (4, 8)], "xk")
            dma_in_grouped(xq_t, xqT, [(0, 4), (4, 8)], "xq")
            wk3 = wk_t[:].rearrange("p (c m) -> p c m", m=CL)
            wq3 = wq_t[:].rearrange("p (c m) -> p c m", m=CL)
            xk3 = xk_t[:].rearrange("p (c n) -> p c n", n=N)
            xq3 = xq_t[:].rearrange("p (c n) -> p c n", n=N)

            def proj_qk(dst_sb, w3, x3, nm):
                for hp in range(2):
                    ps = [qkps.tile([128, 512], F32, tag="ps", name=f"{nm}ps{hp}_{qb}")
                          for qb in range(4)]
                    for cc in range(8):
                        for qb in range(4):
                            nc.tensor.matmul(ps[qb][:],
                                             w3[:, cc, 128 * hp:128 * (hp + 1)],
                                             x3[:, cc, 512 * qb:512 * (qb + 1)],
                                             start=(cc == 0), stop=(cc == 7))
                    for qb in range(4):
                        nc.vector.tensor_copy(
                            dst_sb[:, N * hp + 512 * qb:N * hp + 512 * (qb + 1)],
                            ps[qb][:])

            proj_qk(kT_sb, wk3, xk3, "k")
            proj_qk(qT_sb, wq3, xq3, "q")

        # ---------------- attention-phase pools ----------------
        P_pool = top.enter_context(tc.tile_pool(name="Ppool", bufs=2))
        spool = top.enter_context(tc.tile_pool(name="spool", bufs=2, space="PSUM"))
        popool = top.enter_context(tc.tile_pool(name="popool", bufs=2, space="PSUM"))
        wvpool = top.enter_context(tc.tile_pool(name="wvpool", bufs=1))
        xvpool = top.enter_context(tc.tile_pool(name="xvpool", bufs=1))
        wopool = top.enter_context(tc.tile_pool(name="wopool", bufs=1))
        evpool = top.enter_context(tc.tile_pool(name="evpool", bufs=2))
        dpool = top.enter_context(tc.tile_pool(name="dpool", bufs=2))

        wv_t = wvpool.tile([128, 8 * CL], BF16, tag="wv")
        xv_t = xvpool.tile([128, 8 * N], BF16, tag="xv")
        wo_t = wopool.tile([128, 2 * C], BF16, tag="wo")
        dma_in_grouped(wv_t, wv, [(0, 8)], "wv")
        dma_in_grouped(xv_t, xvT, [(0, 4), (4, 8)], "xv")
        dma_in_grouped(wo_t, wo, [(0, 2)], "wo")
        wv3 = wv_t[:].rearrange("p (c m) -> p c m", m=CL)
        xv3 = xv_t[:].rearrange("p (c n) -> p c n", n=N)
        wo3 = wo_t[:].rearrange("p (c m) -> p c m", m=C)
        nc.vector.memset(v4[:, :, :, D:DV], 1.0)

        # --- filler queue: (cost, min_slot, emit_fn) consumed per slot ---
        fillers = []

        def emit_fillers(slot, budget=3):
            while fillers and budget > 0:
                cost, min_slot, fn = fillers[0]
                if slot < min_slot:
                    return
                fillers.pop(0)
                fn()
                budget -= cost

        def vproj_item(kc):
            def fn():
                ps = popool.tile([128, 512], F32, tag="out", name=f"vps{kc}")
                for cc in range(8):
                    nc.tensor.matmul(ps[:, 0:CL],
                                     xv3[:, cc, 128 * kc:128 * (kc + 1)],
                                     wv3[:, cc, :],
                                     start=(cc == 0), stop=(cc == 7))
                nc.vector.tensor_copy(
                    v4[:, kc, :, 0:D],
                    ps[:, 0:CL].rearrange("p (h d) -> p h d", d=D))
            return (4, 8, fn)   # gated: xv lands mid-u0

        def pv_item(u, Pp, po, par, kc):
            qb, hp = u // 2, u % 2

            def fn():
                nc.tensor.matmul(po[0:DV, :], v4[:, kc, 2 * hp + par, :],
                                 Pp[:, 1024 * kc + 512 * par:
                                    1024 * kc + 512 * (par + 1)],
                                 start=(kc == 0), stop=(kc == NCH - 1))
            return (1, 2, fn)

        def out_item(qb, ev, m):
            def fn():
                ps = popool.tile([128, 512], F32, tag="out", name=f"ops{qb}_{m}")
                for lc in range(2):
                    nc.tensor.matmul(ps[:], wo3[:, lc, 128 * m:128 * (m + 1)],
                                     aT_sb[:, N * lc + 512 * qb:
                                           N * lc + 512 * (qb + 1)],
                                     start=(lc == 0), stop=(lc == 1))
                ev3 = ev[:].rearrange("p (m n) -> p m n", n=512)
                nc.vector.tensor_copy(ev3[:, m, :], ps[:])
            return (2, 2, fn)

        def out_flush(qb, ev):
            def fn():
                nc.sync.dma_start(
                    out=outT.rearrange("(m p) n -> p m n", p=128)
                    [:, :, 512 * qb:512 * (qb + 1)],
                    in_=ev[:].rearrange("p (m n) -> p m n", n=512))
            return (0, 2, fn)

        def emit_scores_unit(u, with_fillers=True):
            qb, hp = u // 2, u % 2
            Pp = P_pool.tile([128, NCH * 1024], BF16, tag="P", name=f"P{u}")
            for kc in range(NCH):
                st = spool.tile([128, 1024], F32, tag="st", name=f"st{u}_{kc}")
                ksl = kT_sb[:, N * hp + 128 * kc:N * hp + 128 * (kc + 1)]
                qsl = qT_sb[:, N * hp + 512 * qb:N * hp + 512 * (qb + 1)]
                nc.tensor.matmul(st[:, 0:512], ksl[0:64, :], qsl[0:64, :],
                                 start=True, stop=True)
                nc.tensor.matmul(st[:, 512:1024], ksl[64:128, :], qsl[64:128, :],
                                 start=True, stop=True)
                nc.scalar.activation(Pp[:, 1024 * kc:1024 * (kc + 1)], st[:],
                                     Exp, scale=float(SCALE))
                if with_fillers:
                    emit_fillers(kc)
            return Pp

        def fin_head(u, po, par):
            qb, hp = u // 2, u % 2
            draw = dpool.tile([1, 512], F32, tag="draw", name=f"dr{u}_{par}")
            drow = dpool.tile([1, 512], F32, tag="drow", name=f"dw{u}_{par}")
            dinv = dpool.tile([64, 512], F32, tag="dinv", name=f"di{u}_{par}")
            nc.vector.tensor_copy(draw[:], po[D:DV, :])
            nc.vector.reciprocal_approx_fast(drow[:], draw[:])
            nc.gpsimd.partition_broadcast(dinv[:], drow[:])
            nc.vector.tensor_mul(
                aT_sb[64 * par:64 * (par + 1),
                      N * hp + 512 * qb:N * hp + 512 * (qb + 1)],
                po[0:D, :], dinv[:])

        # queue V' projection as fillers for scores(u0)/scores(u1) slots
        for kc in range(NCH):
            fillers.append(vproj_item(kc))

        P_cur = emit_scores_unit(0)

        pending_out = None
        for u in range(8):
            qb, hp = u // 2, u % 2
            po_e = popool.tile([128, 512], F32, tag="po", name=f"poe{u}")
            po_o = popool.tile([128, 512], F32, tag="po", name=f"poo{u}")
            if u < 7:
                # PV(u) singles interleaved e,o per kc
                for kc in range(NCH):
                    fillers.append(pv_item(u, P_cur, po_e, 0, kc))
                    fillers.append(pv_item(u, P_cur, po_o, 1, kc))
                if pending_out is not None:
                    oqb, ev = pending_out
                    for m in range(8):
                        fillers.append(out_item(oqb, ev, m))
                    fillers.append(out_flush(oqb, ev))
                    pending_out = None
                P_cur = emit_scores_unit(u + 1)
                # drain any leftover fillers (PV must finish before fin)
                while fillers:
                    fillers.pop(0)[2]()
                fin_head(u, po_e, 0)
                fin_head(u, po_o, 1)
            else:
                # tail: head-even chain, normalize it while head-odd runs
                for kc in range(NCH):
                    pv_item(u, P_cur, po_e, 0, kc)[2]()
                fin_head(u, po_e, 0)
                for kc in range(NCH):
                    pv_item(u, P_cur, po_o, 1, kc)[2]()
                fin_head(u, po_o, 1)
            if hp == 1:
                ev = evpool.tile([128, 8 * 512], BF16, tag="ev", name=f"ev{qb}")
                if u < 7:
                    pending_out = (qb, ev)
                else:
                    for m in range(8):
                        out_item(qb, ev, m)[2]()
                    out_flush(qb, ev)[2]()

    nc.compile()
    return nc


def _get_nc():
    if "nc" not in _CACHE:
        _CACHE["nc"] = build_nc()
    return _CACHE["nc"]


def _make_in_maps(q, k, v, Wq, Wk, Wv, Wo):
    bf = ml_dtypes.bfloat16
    q, k, v = np.asarray(q), np.asarray(k), np.asarray(v)
    Wq, Wk, Wv, Wo = (np.asarray(a) for a in (Wq, Wk, Wv, Wo))
    xqT = [np.ascontiguousarray(q[b].T).astype(bf) for b in range(B)]
    xkT = [np.ascontiguousarray(k[b].T).astype(bf) for b in range(B)]
    xvT = [np.ascontiguousarray(v[b].T).astype(bf) for b in range(B)]
    in_maps = []
    for c in range(8):
        b, g = c // 4, c % 4
        cs = slice(CL * g, CL * (g + 1))
        in_maps.append({
            "xqT": xqT[b], "xkT": xkT[b], "xvT": xvT[b],
            "wq": np.ascontiguousarray(Wq[:, cs]).astype(bf),
            "wk": np.ascontiguousarray(Wk[:, cs]).astype(bf),
            "wv": np.ascontiguousarray(Wv[:, cs]).astype(bf),
            "wo": np.ascontiguousarray(Wo[cs, :]).astype(bf),
        })
    return in_maps


def _run(inputs, trace=False, **kw):
    nc = _get_nc()
    in_maps = _make_in_maps(inputs["q"], inputs["k"], inputs["v"],
                            inputs["Wq"], inputs["Wk"], inputs["Wv"], inputs["Wo"])
    res = None
    for attempt in range(3):
        try:
            res = run_bass_kernel_spmd(nc, in_maps, core_ids=list(range(8)),
                                       trace=trace, **kw)
            break
        except Exception:
            if attempt == 2:
                raise
            import time
            time.sleep(2.0)
    out = np.empty((B, N, C), np.float32)
    for b in range(B):
        acc = res.results[4 * b]["outT"].astype(np.float32)
        for g in range(1, 4):
            acc += res.results[4 * b + g]["outT"].astype(np.float32)
        out[b] = acc.T
    return out, res


def kernel(**inputs) -> np.ndarray:
    out, _ = _run(inputs, trace=False)
    return out
